# revision 28
# baseline (speedup 1.0000x reference)
"""Multi-head attention (B=2, S=2048, E=1024, H=16, D=64) on 8 TRN2 cores.

Sharding: core c handles batch b = c//4 and head-group g = c%4 (4 heads,
256 embed cols). No cross-core communication; host slices inputs (pre-
transposed and pre-cast to bf16) and gathers/normalizes outputs.

Per-core device program (bf16 matmuls, fp32 PSUM accumulation):
  - projections: qhT[c, s] = sum_e wq[e, c] qT[e, s] (c on partitions), so
    attention needs no on-chip transposes; K-bias dropped (softmax-invariant),
    V-bias applied on host (distributes through softmax).
  - attention processes head PAIRS: the two scores matmuls run concurrently
    on disjoint PE row groups (K=64 each) into one [128, 1024] PSUM tile;
    one ScalarE exp (scale=1/8 fused) covers both heads per (iq, jt).
  - vh carries a ones column (m=64), so the out-stage accumulates the
    softmax denominator in PSUM row 64; host divides.
"""

import sys

sys.path.insert(0, "/opt/trn_rl_repo")

import os

import numpy as np

if os.environ.get("JAX_PLATFORMS") == "cpu":
    # the bass program must run on the neuron cores; the axon/neuron PJRT
    # platform registers only when JAX_PLATFORMS is unset/empty
    del os.environ["JAX_PLATFORMS"]

import concourse.bass as bass  # noqa: F401
import concourse.mybir as mybir
from concourse import bacc
from concourse.tile import TileContext

B, S, E = 2, 2048, 1024
H, D = 16, 64
HPC = 4  # heads per core
COLS = HPC * D  # 256
P = 128
F32 = mybir.dt.float32
F16 = mybir.dt.float16
ET = E // P  # 8 e-tiles
JT = S // P  # 16 j-tiles
NB = 512
NIQ = S // NB  # 4 i-quarters

_CACHED = {}


def build():
    nc = bacc.Bacc("TRN2", target_bir_lowering=False, debug=False)
    qT = nc.dram_tensor("qT", [E, S], F16, kind="ExternalInput")
    kT = nc.dram_tensor("kT", [E, S], F16, kind="ExternalInput")
    vT = nc.dram_tensor("vT", [E, S], F16, kind="ExternalInput")
    wq = nc.dram_tensor("wq", [E, COLS], F16, kind="ExternalInput")
    wk = nc.dram_tensor("wk", [E, COLS], F16, kind="ExternalInput")
    wv = nc.dram_tensor("wv", [E, COLS], F16, kind="ExternalInput")
    bq = nc.dram_tensor("bq", [P, 2], F32, kind="ExternalInput")
    # out_raw[:, (h*NIQ+iq)*NB : ...]: rows 0-63 numerator (d), row 64 denom
    out_raw = nc.dram_tensor("out_raw", [65, HPC * S], F32,
                             kind="ExternalOutput")  # [65, 8192]

    with TileContext(nc) as tc:
        with (
            tc.tile_pool(name="wp", bufs=1) as wp,
            tc.tile_pool(name="xq", bufs=ET) as xq,
            tc.tile_pool(name="xk", bufs=ET) as xk,
            tc.tile_pool(name="xv", bufs=ET) as xv,
            tc.tile_pool(name="hp", bufs=1) as hp,
            tc.tile_pool(name="pe", bufs=4) as pe,
            tc.tile_pool(name="psA", bufs=2, space="PSUM") as psA,
            tc.tile_pool(name="psO", bufs=4, space="PSUM") as psO,
        ):
            # --- weights + bias (tiny, first so projections never stall) ---
            wq_b = wp.tile([P, ET, COLS], F16)
            wk_b = wp.tile([P, ET, COLS], F16)
            wv_b = wp.tile([P, ET, COLS], F16)
            nc.sync.dma_start(wq_b, wq.rearrange("(t p) c -> p t c", p=P))
            nc.sync.dma_start(wk_b, wk.rearrange("(t p) c -> p t c", p=P))
            nc.sync.dma_start(wv_b, wv.rearrange("(t p) c -> p t c", p=P))
            bq_t = wp.tile([P, 2], F32)
            nc.sync.dma_start(bq_t, bq[:, :])

            # --- activations, q/k interleaved first, v last ---
            def load_x(pool, dram, tag):
                tiles = []
                for et in range(ET):
                    t = pool.tile([P, S], F16, tag=tag, name=f"{tag}{et}")
                    tiles.append(t)
                return tiles

            qx = load_x(xq, qT, "qx")
            kx = load_x(xk, kT, "kx")
            vx = load_x(xv, vT, "vx")
            # spread the three tensors across independent DMA issue paths so
            # descriptor generation doesn't serialize the loads
            for et in range(ET):
                nc.sync.dma_start(kx[et], kT[et * P : (et + 1) * P, :])
            for et in range(ET):
                nc.scalar.dma_start(vx[et], vT[et * P : (et + 1) * P, :])
            for et in range(ET):
                nc.gpsimd.dma_start(qx[et], qT[et * P : (et + 1) * P, :])

            # --- resident head tensors ---
            qhT = hp.tile([P, 2, S], F16)  # [2 heads x 64 d, chunk, s]
            khT = hp.tile([P, 2, S], F16)
            vh_aug = hp.tile([P, JT, HPC * 65], F16)
            out_sb = hp.tile([P, H // HPC * NIQ * 2, NB], F32)  # [65used, 16, 512]
            nc.vector.memset(vh_aug, 1.0)

            # --- K/Q projections -> transposed head layout [c, s] ---
            # Multiple PSUM slots accumulate in parallel (et outer, sb inner),
            # so consecutive matmuls hit different banks and pipeline at the
            # N-cycle rate instead of serializing on LDWEIGHTS.
            def qk_proj(name, w_b, x, dst, bias, ch, nslots=4):
                """Emit the ch-chunk of a Q/K projection using nslots psum
                slots; with nslots=2 it is split into deferrable chunks."""
                for sb0 in range(0, S // NB, nslots):
                    pss = [
                        psO.tile([P, NB], F32, tag="o", name=f"ps_{name}{ch}{sb0 + j}")
                        for j in range(nslots)
                    ]
                    for eth in range(4):  # 2-et steps = filler chunks
                        for et in range(eth * 2, eth * 2 + 2):
                            for j in range(nslots):
                                nc.tensor.matmul(
                                    pss[j],
                                    w_b[:, et, ch * P : (ch + 1) * P],
                                    x[et][:, (sb0 + j) * NB : (sb0 + j + 1) * NB],
                                    start=(et == 0),
                                    stop=(et == ET - 1),
                                )
                        if eth == 3:  # evacuate before yielding:
                            for j in range(nslots):  # frees slots in-chunk
                                sb = sb0 + j
                                if bias is not None:
                                    nc.vector.tensor_scalar_add(
                                        dst[:, ch, sb * NB : (sb + 1) * NB],
                                        pss[j],
                                        bias[:, ch : ch + 1],
                                    )
                                else:
                                    nc.vector.tensor_copy(
                                        dst[:, ch, sb * NB : (sb + 1) * NB], pss[j]
                                    )
                        yield

            def run_all(gen):
                for _ in gen:
                    pass

            run_all(qk_proj("k", wk_b, kx, khT, None, ch=0))

            # --- V projection -> natural [s, c] layout (no bias) ---
            for sc4 in range(JT // 4):
                pss = [
                    psO.tile([P, NB], F32, tag="o", name=f"ps_v{j}") for j in range(4)
                ]
                for et in range(ET):
                    for j in range(4):
                        sc = sc4 * 4 + j
                        nc.tensor.matmul(
                            pss[j][:, :COLS],
                            vx[et][:, sc * P : (sc + 1) * P],
                            wv_b[:, et, :],
                            start=(et == 0),
                            stop=(et == ET - 1),
                        )
                for j in range(4):
                    sc = sc4 * 4 + j
                    nc.vector.tensor_copy(
                        vh_aug[:, sc].rearrange("p (h x) -> p h x", x=65)[:, :, :D],
                        pss[j][:, :COLS].rearrange("p (h x) -> p h x", x=D),
                    )

            # --- Q projection ch0 (pr=0 attention needs it) ---
            run_all(qk_proj("q", wq_b, qx, qhT, bq_t, ch=0))

            # ch1 projections stream into attention pair 0's PE slack as
            # ~8-matmul chunks (2 psum slots so op0/op1 keep the other two).
            # Each generator emits one chunk per next(); consumed lazily.
            fillers = [
                qk_proj("k", wk_b, kx, khT, None, ch=1, nslots=2),
                qk_proj("q", wq_b, qx, qhT, bq_t, ch=1, nslots=2),
            ]

            def emit_filler_chunk():
                while fillers:
                    try:
                        next(fillers[0])
                        return True
                    except StopIteration:
                        fillers.pop(0)
                return False

            # --- attention, head pairs; software-pipelined emission ---
            # Per step: scores(t) + exp(t) go FIRST so ScalarE always has the
            # next exp queued; out(t-1) and small projection filler chunks run
            # in PE's slack while exp(t) executes.
            steps = [(pr, iq, jt) for pr in range(2) for iq in range(NIQ)
                     for jt in range(JT)]
            ops = {}  # (pr, iq) -> (op0, op1)
            prev = None  # (pr, iq, jt, expT)

            def emit_out(pr, iq, jt, expT):
                op0, op1 = ops[(pr, iq)]
                for hh, op in ((0, op0), (1, op1)):
                    h = 2 * pr + hh
                    nc.tensor.matmul(
                        op[:65, :],
                        vh_aug[:, jt, h * 65 : (h + 1) * 65],
                        expT[:, hh * NB : (hh + 1) * NB],
                        start=(jt == 0),
                        stop=(jt == JT - 1),
                    )
                if jt == JT - 1:  # evacuate + store this iq's outputs
                    for hh, op in ((0, op0), (1, op1)):
                        r = (2 * pr + hh) * NIQ + iq
                        nc.vector.tensor_copy(out_sb[:65, r, :], op[:65, :])
                        nc.sync.dma_start(
                            out_raw[:, r * NB : (r + 1) * NB], out_sb[:65, r, :]
                        )
                    del ops[(pr, iq)]

            for it, (pr, iq, jt) in enumerate(steps):
                if jt == 0:
                    ops[(pr, iq)] = (
                        psO.tile([P, NB], F32, tag="o", name="op0"),
                        psO.tile([P, NB], F32, tag="o", name="op1"),
                    )
                sps = psA.tile([P, 1024], F32, tag="s", name="sps")
                for hh in range(2):  # row-group packed, concurrent
                    r0 = hh * D
                    nc.tensor.matmul(
                        sps[:, hh * NB : (hh + 1) * NB],
                        khT[r0 : r0 + D, pr, jt * P : (jt + 1) * P],
                        qhT[r0 : r0 + D, pr, iq * NB : (iq + 1) * NB],
                        start=True,
                        stop=True,
                    )
                expT = pe.tile([P, 1024], F16, tag="e", name="expT")
                nc.scalar.activation(
                    expT, sps, mybir.ActivationFunctionType.Exp, scale=0.125
                )
                if prev is not None:
                    emit_out(*prev)
                if it % 16 in (3, 7, 11, 15):
                    emit_filler_chunk()
                prev = (pr, iq, jt, expT)
            emit_out(*prev)
    nc.finalize()
    return nc


def _prep_in_maps(q, k, v, wq, bq, wk, bk, wv, bv):
    bf = np.float16
    q, k, v = (np.asarray(x, np.float32) for x in (q, k, v))
    wqb, wkb, wvb = (np.asarray(x, bf) for x in (wq, wk, wv))
    bq = np.asarray(bq, np.float32)
    qT = [np.ascontiguousarray(q[b].T.astype(bf)) for b in range(B)]
    kT = [np.ascontiguousarray(k[b].T.astype(bf)) for b in range(B)]
    vT = [np.ascontiguousarray(v[b].T.astype(bf)) for b in range(B)]
    in_maps = []
    for c in range(8):
        b, g = divmod(c, 4)
        cs = slice(g * COLS, (g + 1) * COLS)
        in_maps.append(
            {
                "qT": qT[b],
                "kT": kT[b],
                "vT": vT[b],
                "wq": np.ascontiguousarray(wqb[:, cs]),
                "wk": np.ascontiguousarray(wkb[:, cs]),
                "wv": np.ascontiguousarray(wvb[:, cs]),
                "bq": np.ascontiguousarray(bq[cs].reshape(2, P).T),
            }
        )
    return in_maps


def _make_runner(nc, n_cores=8):
    """Persistent jitted shard_map runner over the prebuilt Bass module."""
    import jax
    from jax.experimental.shard_map import shard_map
    from jax.sharding import Mesh, NamedSharding, PartitionSpec
    from concourse import bass2jax

    bass2jax.install_neuronx_cc_hook()

    in_names, out_names, out_avals, zero_outs = [], [], [], []
    for alloc in nc.m.functions[0].allocations:
        if not isinstance(alloc, mybir.MemoryLocationSet):
            continue
        name = alloc.memorylocations[0].name
        if alloc.kind == "ExternalInput":
            in_names.append(name)
        elif alloc.kind == "ExternalOutput":
            shape = tuple(alloc.tensor_shape)
            dtype = mybir.dt.np(alloc.dtype)
            out_avals.append(jax.core.ShapedArray(shape, dtype))
            zero_outs.append(np.zeros((n_cores * shape[0], *shape[1:]), dtype))
            out_names.append(name)
    pid_name = nc.partition_id_tensor.name if nc.partition_id_tensor else None
    if pid_name is not None:
        in_names = [n for n in in_names if n != pid_name]
    n_params = len(in_names)
    all_names = in_names + out_names + ([pid_name] if pid_name else [])

    def _body(*args):
        operands = list(args)
        if pid_name is not None:
            operands.append(bass2jax.partition_id_tensor())
        outs = bass2jax._bass_exec_p.bind(
            *operands,
            out_avals=tuple(out_avals),
            in_names=tuple(all_names),
            out_names=tuple(out_names),
            lowering_input_output_aliases=(),
            sim_require_finite=True,
            sim_require_nnan=True,
            nc=nc,
        )
        return tuple(outs)

    devices = jax.devices()[:n_cores]
    mesh = Mesh(np.asarray(devices), ("core",))
    nio = n_params + len(out_names)
    sharded = jax.jit(
        shard_map(
            _body,
            mesh=mesh,
            in_specs=(PartitionSpec("core"),) * nio,
            out_specs=(PartitionSpec("core"),) * len(out_names),
            check_rep=False,
        ),
        keep_unused=True,
    )
    row_sharding = NamedSharding(mesh, PartitionSpec("core"))
    zeros_dev = [jax.device_put(z, row_sharding) for z in zero_outs]

    def run(in_maps):
        concat_in = [
            np.concatenate([np.asarray(m[name]) for m in in_maps], axis=0)
            for name in in_names
        ]
        out_arrs = sharded(*concat_in, *zeros_dev)
        return [
            {
                name: np.asarray(out_arrs[i]).reshape(n_cores, *out_avals[i].shape)[c]
                for i, name in enumerate(out_names)
            }
            for c in range(n_cores)
        ]

    run.sharded = sharded
    run.in_names = in_names
    run.zeros_dev = zeros_dev
    run.row_sharding = row_sharding
    return run


def get_runner():
    if "run" not in _CACHED:
        _CACHED["nc"] = build()
        _CACHED["run"] = _make_runner(_CACHED["nc"])
    return _CACHED["run"]


def kernel(q, k, v, wq, bq, wk, bk, wv, bv):
    run = get_runner()
    in_maps = _prep_in_maps(q, k, v, wq, bq, wk, bk, wv, bv)
    results = run(in_maps)

    bv = np.asarray(bv, np.float32)
    out = np.empty((B, S, E), np.float32)
    for c in range(8):
        b, g = divmod(c, 4)
        raw = results[c]["out_raw"]  # [65, 8192]
        num = raw[:64].reshape(64, HPC, S)  # [d, h, i] (NIQ*NB = S)
        den = raw[64].reshape(HPC, S)
        for h in range(HPC):
            col0 = g * COLS + h * D
            o = num[:, h, :] / den[h][None, :]
            out[b, :, col0 : col0 + D] = o.T + bv[col0 : col0 + D][None, :]
    return out
